# revision 62
# baseline (speedup 1.0000x reference)
"""Trainium2 Bass kernel for nn_Encoder_29454885716713.

Reference computation (per batch b of B=32, S=2048, F=64):
    q = x @ Wq; k = x @ Wk; v = x @ Wv
    a = softmax(q @ k.T, axis=0 over q)       # query-axis softmax
    out = (a @ v) @ Wh

Sharding: data-parallel over batch, 4 batches per core x 8 cores.

Kernel strategy (per core):
  - Host packs a blob [128, 4256]: Wh, G = Wq @ Wk.T (folded on host, so
    scores = x G x^T need ONE projection y = x G instead of q and k),
    Wv, and the 4 pre-transposed xT batches two-per-row-half.  Input
    DMAs are split (weights / xT b0 / xT b1 / xT b2+b3) so compute
    starts after ~weights+one-batch of traffic.
  - yT via 4 matmuls contracting F=64 partitions (tile_position output
    offset keeps y in the same partition half as its xT so score
    matmuls can contract over that half); v in natural layout [k',d].
  - Per k'-tile t: aT_t = [128, S] in PSUM via matmuls with xT tile as
    stationary; ScalarE exp with fused accum_out emits exp(aT_t) plus
    the query-axis softmax denominator Z[k'] (free-dim reduction in
    this layout; fp32 range makes max-subtraction unnecessary).
  - 1/Z folded into v rows; h1 accumulates in four persistent PSUM
    banks, interleaved tile-by-tile into the exp chain; out = h1 @ Wh.
  - Batch b+1's projections are issued mid-way through batch b's score
    loop (PSUM slot reader-tracking keeps the WAR fences tile-local),
    and batch b's h1-drain/output stage is deferred to tile 1 of batch
    b+1's loop, so ScalarE never idles at batch boundaries.
  - This walrus build allows only ONE sync-wait slot per ISA
    instruction.  Tiny per-engine "absorber" ops (dummy matmul / copy /
    nop), each carrying exactly one cross-engine wait, precede any
    instruction that would otherwise need two.
"""

import numpy as np

_CACHE = {}

B, S, F = 32, 2048, 64
DQ, DK, DV = 24, 24, 32
NCORES = 8
BPC = B // NCORES
NT = S // 128
NQC = S // 512

C_WHR = 0
C_G = 64   # 128 cols: [G|0] in the top row-half, [0|G] in the bottom,
           # so the f32r projection matmul gets the 128-col stationary
           # tile the ISA requires and y lands in its batch's partition
           # half via the zero block
C_WV = 192
C_XT = 224
BLOB_COLS = C_XT + (BPC // 2) * S  # 4320
PREFETCH_T = 10  # 6 projection steps pop one-per-tile at t=10..15


def _build(lowering=True):
    import concourse.bass as bass
    import concourse.mybir as mybir
    import concourse.tile as tile
    from concourse.bass import _add_dep_helper

    f32 = mybir.dt.float32
    f32r = mybir.dt.float32r
    bf16 = mybir.dt.bfloat16
    EXPF = mybir.ActivationFunctionType.Exp

    def r(ap):
        return ap.bitcast(f32r)

    nc = bass.Bass(target_bir_lowering=lowering)
    blob_h = nc.dram_tensor("blob", [128, BLOB_COLS], f32r, kind="ExternalInput")
    out_h = nc.dram_tensor("out", [BPC, S, F], f32, kind="ExternalOutput")
    out_d = out_h.ap()

    with tile.TileContext(nc) as tc:
        with (
            tc.tile_pool(name="consts", bufs=1) as consts,
            tc.tile_pool(name="yy", bufs=2) as y_pool,
            tc.tile_pool(name="vnat", bufs=2) as vnat_pool,
            tc.tile_pool(name="ea", bufs=16) as ea_pool,
            tc.tile_pool(name="zz", bufs=64) as z_pool,
            tc.tile_pool(name="vs", bufs=16) as vs_pool,
            tc.tile_pool(name="h1c", bufs=4) as h1c_pool,
            tc.tile_pool(name="ob", bufs=2) as ob_pool,
            tc.tile_pool(name="scr", bufs=1) as scr_pool,
            tc.tile_pool(name="pa", bufs=2, space="PSUM") as pa_pool,
            tc.tile_pool(name="php", bufs=1, space="PSUM") as php_pool,
        ):
            blob_sb = consts.tile([128, BLOB_COLS], f32r)
            sp = C_XT + S
            hm_ = C_XT + 1024
            dmaA = nc.sync.dma_start(
                out=blob_sb[:, 0:C_XT], in_=blob_h.ap()[:, 0:C_XT]
            )
            dmaB1 = nc.sync.dma_start(
                out=blob_sb[0:64, C_XT:hm_], in_=blob_h.ap()[0:64, C_XT:hm_]
            )
            dmaB2 = nc.sync.dma_start(
                out=blob_sb[0:64, hm_:sp], in_=blob_h.ap()[0:64, hm_:sp]
            )
            dmaC = nc.gpsimd.dma_start(
                out=blob_sb[64:128, C_XT:sp], in_=blob_h.ap()[64:128, C_XT:sp]
            )
            dmaD = nc.gpsimd.dma_start(
                out=blob_sb[:, sp:BLOB_COLS], in_=blob_h.ap()[:, sp:BLOB_COLS]
            )
            wh_sb = blob_sb[0:DV, 0:F]

            # ---------- absorber machinery ----------
            php = php_pool.tile([128, 4, 512], f32)
            dve_scr = scr_pool.tile([1, 256], f32)
            act_scr = scr_pool.tile([1, 256], f32)
            # dedicated operand tile for absorber dummy matmuls: reading
            # blob_sb would pick up a tile-granularity auto-dep on the
            # LAST blob DMA, gating early absorbers on the full load
            mat_scr = scr_pool.tile([128, 16], f32)
            nc.vector.memset(mat_scr, 0.0)
            ctr = {"pe": 0, "dve": 0, "act": 0}

            def pe_absorb(producer):
                c = ctr["pe"]; ctr["pe"] = c + 1
                d = nc.tensor.matmul(
                    php[64:64 + DQ, c % 4, 2 * (c // 4):2 * (c // 4) + 2],
                    mat_scr[64:128, 0:12].bitcast(bf16),
                    mat_scr[64:128, 0:1].bitcast(bf16),
                    start=True, stop=True, skip_group_check=True,
                    tile_position=(64, 64),
                )
                if producer is not None:
                    _add_dep_helper(d.ins, producer.ins, True, "absorb")
                return d

            def dve_absorb(producer):
                c = ctr["dve"] % 250; ctr["dve"] += 1
                d = nc.vector.memset(dve_scr[:, c + 1:c + 2], 0.0)
                _add_dep_helper(d.ins, producer.ins, True, "absorb")
                return d

            def act_absorb(producer):
                c = ctr["act"] % 250; ctr["act"] += 1
                d = nc.scalar.copy(act_scr[:, c + 1:c + 2], act_scr[:, 0:1])
                if producer is not None:
                    _add_dep_helper(d.ins, producer.ins, True, "absorb")
                return d

            def order(after, before):
                _add_dep_helper(after.ins, before.ins, False, "order")

            wfA = pe_absorb(dmaA)
            # wfC/wfD are created lazily at first use: an upfront absorber
            # would park the in-order PE queue on the DMA it waits for
            xdma = {1: dmaC, 2: dmaD, 3: dmaD}
            xfence_cache = {}

            def xfence(b):
                if b not in xfence_cache:
                    f = pe_absorb(xdma[b])
                    xfence_cache[b] = f
                    if b == 2:
                        xfence_cache[3] = f
                return xfence_cache[b]

            nc.vector.memset(act_scr[:, 0:1], 0.0)
            act_absorb(None)  # ACT observes the act_scr init (DVE) once

            # PSUM "pa" slot tracking: 2 rotating [128,1024] buffers; the
            # FIRST matmul into a slot carries the WAR wait on the slot's
            # previous reader (exp / DVE copy) directly — a real
            # instruction needs no absorber and, unlike the absorber
            # dummy, writes no php (whose tile-granular WAR tracking
            # would drag in unrelated h1-drain copies).
            pa_state = {"idx": 0, "readers": [None, None], "dve_pending": None}

            def pa_alloc(latest=False):
                # per-slot fencing keeps PE two exps behind ACT; but the
                # pool's DVE hazard tracking is coarse (lands on the
                # LATEST DVE reader of any pa tile), so when a DVE reader
                # is still unabsorbed a second absorber covers it
                i = pa_state["idx"]; pa_state["idx"] = i + 1
                slot = i % 2
                producer = pa_state["readers"][slot]
                fence = pe_absorb(producer) if producer is not None else None
                if pa_state["dve_pending"] is not None:
                    if pa_state["dve_pending"] is not producer:
                        pe_absorb(pa_state["dve_pending"])
                    pa_state["dve_pending"] = None
                t = pa_pool.tile([128, 1024], f32, tag="pa")
                return t, fence, slot

            def pa_set_reader(slot, reader, dve=False):
                pa_state["readers"][slot] = reader
                if dve:
                    pa_state["dve_pending"] = reader

            def project(b):
                """y = x G and v = x Wv for batch b, split into 6 steps
                (one pa slot each) so a prefetch can be spread across the
                previous batch's score loop."""
                rb = (b % 2) * 64
                x0 = C_XT + (b // 2) * S
                xT = blob_sb[rb:rb + 64, x0:x0 + S]
                g_sb = blob_sb[rb:rb + 64, C_G:C_G + 128]
                wv_sb = blob_sb[rb:rb + 64, C_WV:C_WV + DV]
                wf = xfence(b)
                yT = y_pool.tile([128, S], f32)
                vnat = vnat_pool.tile([128, NT * DV], f32)
                state = {"rb": rb, "x0": x0, "xT": xT, "yT": yT,
                         "vnat": vnat, "last_y_cp": None}

                def y_step(qc):
                    sl = slice(qc * 512, (qc + 1) * 512)
                    p, fence, slot = pa_alloc()
                    mm = nc.tensor.matmul(
                        p[:, 0:512], g_sb, xT[:, sl],
                        start=True, stop=True, tile_position=(rb, 0),
                    )
                    order(mm, wf)
                    order(mm, wfA)
                    if fence is not None:
                        order(mm, fence)
                    da = dve_absorb(mm)
                    cp = nc.vector.tensor_copy(
                        r(yT[rb:rb + 64, sl]), p[rb:rb + 64, 0:512]
                    )
                    order(cp, da)
                    pa_set_reader(slot, cp, dve=True)
                    state["last_y_cp"] = cp

                def v_step(g):
                    p, fence, slot = pa_alloc()
                    mm = None
                    for i in range(8):
                        t = 8 * g + i
                        mm = nc.tensor.matmul(
                            p[:, i * DV:(i + 1) * DV],
                            xT[:, t * 128:(t + 1) * 128], wv_sb,
                            start=True, stop=True, tile_position=(rb, 0),
                        )
                        if i == 0:
                            order(mm, wf)
                            if fence is not None:
                                order(mm, fence)
                    da = dve_absorb(mm)
                    cp = nc.vector.tensor_copy(
                        vnat[:, g * 8 * DV:(g + 1) * 8 * DV], p[:, 0:8 * DV]
                    )
                    order(cp, da)
                    pa_set_reader(slot, cp, dve=True)
                    state["last_v_cp"] = cp

                def fence_step():
                    # raise PE's DVE-sem watermark past this projection's
                    # copies, so the next batch's t0 score matmuls carry
                    # no extra wait for yT (their pa-slot WAR is the one
                    # allowed wait).  Issued while php WAR deps are
                    # long-satisfied, unlike a fence at loop start which
                    # would pick up the h1-drain copies via the absorber
                    # dummy's php write.
                    state["yfence"] = pe_absorb(state["last_v_cp"])

                steps = [lambda qc=qc: y_step(qc) for qc in range(NQC)]
                steps += [lambda g=g: v_step(g) for g in range(2)]
                steps.append(fence_step)
                return state, steps

            def cold_project():
                """Batch-0 projection on the critical startup path: y qc
                pairs share one pa tile (two 512-wide matmuls + a single
                1024-wide copy), and work is ordered after the half of
                the xT DMA it actually needs."""
                rb = 0
                x0 = C_XT
                xT = blob_sb[0:64, x0:x0 + S]
                g_sb = blob_sb[0:64, C_G:C_G + 128]
                wv_sb = blob_sb[0:64, C_WV:C_WV + DV]
                yT = y_pool.tile([128, S], f32)
                vnat = vnat_pool.tile([128, NT * DV], f32)
                state = {"rb": rb, "x0": x0, "xT": xT, "yT": yT,
                         "vnat": vnat, "last_y_cp": None, "last_v_cp": None}

                def y_cold(qpair, wfx):
                    p, fence, slot = pa_alloc()
                    mm = None
                    for j, qc in enumerate(qpair):
                        sl = slice(qc * 512, (qc + 1) * 512)
                        mm = nc.tensor.matmul(
                            p[:, j * 512:(j + 1) * 512], g_sb, xT[:, sl],
                            start=True, stop=True, tile_position=(0, 0),
                        )
                        if j == 0:
                            order(mm, wfA)
                            order(mm, wfx)
                            if fence is not None:
                                order(mm, fence)
                    da = dve_absorb(mm)
                    sl2 = slice(qpair[0] * 512, (qpair[-1] + 1) * 512)
                    cp = nc.vector.tensor_copy(
                        r(yT[0:64, sl2]), p[0:64, 0:1024]
                    )
                    order(cp, da)
                    pa_set_reader(slot, cp, dve=True)
                    state["last_y_cp"] = cp

                def v_cold(g, wfx):
                    p, fence, slot = pa_alloc()
                    mm = None
                    for i in range(8):
                        t = 8 * g + i
                        mm = nc.tensor.matmul(
                            p[:, i * DV:(i + 1) * DV],
                            xT[:, t * 128:(t + 1) * 128], wv_sb,
                            start=True, stop=True, tile_position=(0, 0),
                        )
                        if i == 0:
                            order(mm, wfx)
                            if fence is not None:
                                order(mm, fence)
                    da = dve_absorb(mm)
                    cp = nc.vector.tensor_copy(
                        vnat[:, g * 8 * DV:(g + 1) * 8 * DV], p[:, 0:8 * DV]
                    )
                    order(cp, da)
                    pa_set_reader(slot, cp, dve=True)
                    state["last_v_cp"] = cp

                wfB1 = pe_absorb(dmaB1)
                y_cold((0, 1), wfB1)
                v_cold(0, wfB1)  # everything above needs only dmaB1's half
                wfB2 = pe_absorb(dmaB2)
                y_cold((2, 3), wfB2)
                v_cold(1, wfB2)
                state["yfence"] = pe_absorb(state["last_v_cp"])
                return state

            st = {"prev_dve": None, "last_h1cp": None,
                  "last_mm": None, "last_dve": None}
            pending_s4 = []
            prev_batch_exp = None
            out_dmas = []

            def s4_steps(b, ea_tiles, vs_tiles, vs_muls, last_exp, last_vs, ob):
                """h1 drain + out for batch b, split into 3 steps so the
                PE/DVE ladder pipelines under the next batch's exps:
                step 0: all 4 h1-final matmuls + h1 copies (no pa use);
                step 1: out matmuls + obuf copies + DMA for qc 0,1;
                step 2: same for qc 2,3."""
                obuf = ob_pool.tile([128, NT * F], f32)
                h1cats = [None] * NQC

                def drain_step():
                    # all 4 stop-matmuls first, then all 4 copies: an
                    # interleaved mm/copy sequence serializes via php's
                    # tile-granular WAR (mm qc+1 waits copy qc)
                    hfence_a = pe_absorb(last_exp)
                    hfence_d = pe_absorb(last_vs)
                    lastmm = None
                    for qc in range(NQC):
                        sl = slice(qc * 512, (qc + 1) * 512)
                        lastmm = nc.tensor.matmul(
                            php[0:DV, qc, :],
                            vs_tiles[NT - 1],
                            ea_tiles[NT - 1][:, sl],
                            start=False, stop=True,
                            skip_group_check=True,
                        )
                        order(lastmm, hfence_a)
                        order(lastmm, hfence_d)
                    da = dve_absorb(lastmm)
                    for qc in range(NQC):
                        h1cat = h1c_pool.tile([DV, 512], f32r)
                        h1copy = nc.vector.tensor_copy(h1cat, php[0:DV, qc, :])
                        if qc == 0:
                            order(h1copy, da)
                        h1cats[qc] = h1cat
                        st["last_h1cp"] = h1copy
                    # raise PE's DVE watermark past the h1 copies so later
                    # absorber dummies don't carry the coarse php WAR as a
                    # second sem wait; reused as out_step's ofence so the
                    # scheduler can't order the out matmuls before it
                    st["h1wm"] = pe_absorb(st["last_h1cp"])

                def out_step(qcs):
                    ocp = None
                    for qc in qcs:
                        ofence = st.get("h1wm") if qc == qcs[0] else None
                        pout, pfence, slot = pa_alloc()
                        lastmm = None
                        for si in range(4):
                            lastmm = nc.tensor.matmul(
                                pout[:, si * F:(si + 1) * F],
                                h1cats[qc][:, si * 128:(si + 1) * 128],
                                wh_sb,
                                start=True, stop=True,
                            )
                            if si == 0 and pfence is not None:
                                order(lastmm, pfence)
                            if ofence is not None:
                                order(lastmm, ofence)
                        da = dve_absorb(lastmm)
                        if len(out_dmas) >= 4:
                            # obuf slot reuse: absorb the matching
                            # out-DMA from two batches ago
                            da2 = dve_absorb(out_dmas[-4])
                        else:
                            da2 = None
                        ocp = nc.vector.tensor_copy(
                            obuf[:, qc * 4 * F:(qc + 1) * 4 * F],
                            pout[:, 0:4 * F],
                        )
                        order(ocp, da)
                        if da2 is not None:
                            order(ocp, da2)
                        pa_set_reader(slot, ocp, dve=True)
                        st["prev_dve"] = ocp
                    # one DMA per half-batch on alternating queues keeps
                    # total DMA count within the 16 hardware rings (a 17th
                    # DMA would inherit a ring-reuse wait on top of its
                    # copy dependency)
                    obv = obuf.rearrange("p (t f) -> p t f", f=F)
                    q0, q1 = qcs[0] * 4, (qcs[-1] + 1) * 4
                    eng = nc.sync if qcs[0] == 0 else nc.gpsimd
                    odma = eng.dma_start(
                        out=ob[:, q0:q1, :], in_=obv[:, q0:q1, :],
                    )
                    out_dmas.append(odma)

                return [drain_step,
                        lambda: out_step((0, 1)),
                        lambda: out_step((2, 3))]

            proj = cold_project()
            next_proj = None
            for b in range(BPC):
                ob = out_d[b].rearrange("(t p) f -> p t f", p=128)
                rb, xT, yT, vnat = proj["rb"], proj["xT"], proj["yT"], proj["vnat"]

                # ---- aT + exp + Z ----
                qfence = proj.get("yfence")
                if qfence is None:
                    qfence = pe_absorb(proj["last_v_cp"])
                # ACT self-sem watermark: absorbs the previous batch's ea
                # WAW so each exp keeps a single (PE) wait
                bfence = (
                    act_absorb(prev_batch_exp)
                    if prev_batch_exp is not None else None
                )
                prefetch_steps = []
                ea_tiles = []
                vs_tiles = []
                last_exp = None
                last_exp_h0 = None
                last_vs = None
                vs_muls = [None] * NT
                for t in range(NT):
                    if 1 <= t <= 3 and pending_s4:
                        pending_s4.pop(0)()
                    if t == PREFETCH_T and b + 1 < BPC:
                        next_proj, prefetch_steps = project(b + 1)
                    if prefetch_steps and (t - PREFETCH_T) % 2 == 0:
                        # pairs keep the 2-slot pa ring parity even, so
                        # score matmuls keep fencing on the exp from two
                        # slots back rather than the immediately
                        # preceding one
                        prefetch_steps.pop(0)()
                        if prefetch_steps:
                            prefetch_steps.pop(0)()
                    elif prefetch_steps and t == NT - 1:
                        prefetch_steps.pop(0)()  # trailing fence step
                    ea = ea_pool.tile([128, S], f32r, tag="ea")
                    zp = z_pool.tile([128, 2], f32, tag="zp")
                    for h in range(2):
                        pa, afence, slot = pa_alloc()
                        lastmm = None
                        for j in range(2):
                            qc = 2 * h + j
                            lastmm = nc.tensor.matmul(
                                pa[:, j * 512:(j + 1) * 512],
                                xT[:, t * 128:(t + 1) * 128],
                                r(yT[rb:rb + 64, qc * 512:(qc + 1) * 512]),
                                start=True, stop=True, tile_position=(rb, 0),
                            )
                            if j == 0 and afence is not None:
                                order(lastmm, afence)
                            if t == 0:
                                order(lastmm, qfence)
                        if h == 1 and t >= 1:
                            # interleave h1 accumulation for tile t-1
                            vf = pe_absorb(vs_muls[t - 1])
                            hcf = (
                                pe_absorb(st["last_h1cp"])
                                if t == 1 and st["last_h1cp"] is not None
                                else None
                            )
                            for qc2 in range(4):
                                hm = nc.tensor.matmul(
                                    php[0:DV, qc2, :],
                                    vs_tiles[t - 1],
                                    ea_tiles[t - 1][:, qc2 * 512:(qc2 + 1) * 512],
                                    start=(t - 1 == 0), stop=False,
                                    skip_group_check=True,
                                )
                                order(hm, vf)
                                if hcf is not None:
                                    order(hm, hcf)
                        last_exp = nc.scalar.activation(
                            out=ea[:, h * 1024:(h + 1) * 1024],
                            in_=pa,
                            func=EXPF,
                            accum_out=zp[:, h:h + 1],
                        )
                        pa_set_reader(slot, last_exp)
                        if h == 0:
                            last_exp_h0 = last_exp
                        st["last_mm"] = lastmm
                        if bfence is not None:
                            order(last_exp, bfence)
                    zs = z_pool.tile([128, 1], f32, tag="zs")
                    nc.vector.tensor_add(zs, zp[:, 0:1], zp[:, 1:2])
                    zi = z_pool.tile([128, 1], f32, tag="zi")
                    nc.vector.reciprocal(zi, zs)
                    vs = vs_pool.tile([128, DV], f32r, tag="vs")
                    last_vs = nc.vector.tensor_scalar_mul(
                        vs, vnat[:, t * DV:(t + 1) * DV], zi
                    )
                    st["last_dve"] = last_vs
                    ea_tiles.append(ea)
                    vs_tiles.append(vs)
                    vs_muls[t] = last_vs
                prev_batch_exp = last_exp

                pending_s4.extend(
                    s4_steps(b, ea_tiles, vs_tiles, vs_muls, last_exp, last_vs, ob)
                )
                proj = next_proj

            while pending_s4:
                pending_s4.pop(0)()
            # ---- tail: sync-nop chain so the auto drain keeps <=1 wait ----
            for fin in [st["prev_dve"], prev_batch_exp,
                        dmaA, dmaB1, dmaB2, dmaC, dmaD] + out_dmas:
                if fin is None:
                    continue
                n = nc.sync.nop()
                _add_dep_helper(n.ins, fin.ins, True, "drain pre-wait")
    return nc


def _get_nc():
    if "nc" not in _CACHE:
        _CACHE["nc"] = _build()
    return _CACHE["nc"]


def make_in_maps(x, Wq, Wk, Wv, Wh):
    x = np.asarray(x, dtype=np.float32)
    xt = np.ascontiguousarray(x.transpose(0, 2, 1))  # [B, F, S]
    g = np.asarray(Wq, dtype=np.float32) @ np.asarray(Wk, dtype=np.float32).T
    wv = np.asarray(Wv, dtype=np.float32)
    base = np.zeros((128, BLOB_COLS), dtype=np.float32)
    base[0:DV, 0:F] = np.asarray(Wh, dtype=np.float32)
    for rb in (0, 64):
        base[rb:rb + 64, C_G + rb:C_G + rb + 64] = g
        base[rb:rb + 64, C_WV:C_WV + DV] = wv
    maps = []
    for i in range(NCORES):
        blob = base.copy()
        for b in range(BPC):
            rb = (b % 2) * 64
            x0 = C_XT + (b // 2) * S
            blob[rb:rb + 64, x0:x0 + S] = xt[i * BPC + b]
        maps.append({"blob": blob})
    return maps


def _get_exec():
    """Compile once and cache the jitted shard_map(bass_exec) callable so
    repeated kernel() calls skip retracing/recompilation."""
    if "exec" in _CACHE:
        return _CACHE["exec"]
    import jax
    from jax.sharding import Mesh, NamedSharding, PartitionSpec
    from concourse import bass2jax, mybir

    nc = _get_nc()
    bass2jax.install_neuronx_cc_hook()
    partition_name = (
        nc.partition_id_tensor.name if nc.partition_id_tensor else None
    )
    in_names, out_names, out_avals = [], [], []
    for alloc in nc.m.functions[0].allocations:
        if not isinstance(alloc, mybir.MemoryLocationSet):
            continue
        name = alloc.memorylocations[0].name
        if alloc.kind == "ExternalInput":
            if name != partition_name:
                in_names.append(name)
        elif alloc.kind == "ExternalOutput":
            out_names.append(name)
            out_avals.append(
                jax.ShapeDtypeStruct(
                    tuple(alloc.tensor_shape), mybir.dt.np(alloc.dtype)
                )
            )
    all_in_names = list(in_names) + list(out_names)
    if partition_name is not None:
        all_in_names.append(partition_name)

    def _body(*args):
        operands = list(args)
        if partition_name is not None:
            operands.append(bass2jax.partition_id_tensor())
        return tuple(
            bass2jax._bass_exec_p.bind(
                *operands,
                out_avals=tuple(
                    jax.core.ShapedArray(a.shape, a.dtype) for a in out_avals
                ),
                in_names=tuple(all_in_names),
                out_names=tuple(out_names),
                lowering_input_output_aliases=(),
                sim_require_finite=True,
                sim_require_nnan=True,
                nc=nc,
            )
        )

    devices = jax.devices()[:NCORES]
    mesh = Mesh(np.asarray(devices), ("core",))
    n_params = len(in_names)
    n_outs = len(out_names)
    in_specs = (PartitionSpec("core"),) * (n_params + n_outs)
    out_specs = (PartitionSpec("core"),) * n_outs
    sh = NamedSharding(mesh, PartitionSpec("core"))

    try:
        sm = jax.shard_map(
            _body, mesh=mesh, in_specs=in_specs, out_specs=out_specs,
            check_vma=False,
        )
    except TypeError:
        from jax.experimental.shard_map import shard_map as _sm

        sm = _sm(
            _body, mesh=mesh, in_specs=in_specs, out_specs=out_specs,
            check_rep=False,
        )
    jitted = jax.jit(
        sm,
        donate_argnums=tuple(range(n_params, n_params + n_outs)),
        keep_unused=True,
    )
    lower_args = [
        jax.ShapeDtypeStruct((NCORES * 128, BLOB_COLS), np.float32, sharding=sh)
    ] + [
        jax.ShapeDtypeStruct(
            (NCORES * a.shape[0], *a.shape[1:]), a.dtype, sharding=sh
        )
        for a in out_avals
    ]

    def compile_fn():
        return jitted.lower(*lower_args).compile()

    fast = bass2jax.fast_dispatch_compile(compile_fn)
    _CACHE["exec"] = {
        "fast": fast,
        "sh": sh,
        "out_avals": out_avals,
        "donate_next": None,
    }
    return _CACHE["exec"]


def kernel(x, Wq, Wk, Wv, Wh):
    import jax

    in_maps = make_in_maps(x, Wq, Wk, Wv, Wh)
    try:
        ex = _get_exec()
        blob_glob = np.concatenate([m["blob"] for m in in_maps], axis=0)
        blob_dev = jax.device_put(blob_glob, ex["sh"])
        donate = ex["donate_next"]
        if donate is None:
            donate = [
                jax.device_put(
                    np.zeros((NCORES * a.shape[0], *a.shape[1:]), a.dtype),
                    ex["sh"],
                )
                for a in ex["out_avals"]
            ]
        outs = ex["fast"](blob_dev, *donate)
        aval = ex["out_avals"][0]
        out_np = np.asarray(outs[0]).reshape(NCORES, *aval.shape)
        # previous outputs become the next call's donated buffers (the
        # kernel fully overwrites them)
        ex["donate_next"] = list(outs)
        return np.concatenate([out_np[c] for c in range(NCORES)], axis=0)
    except Exception:
        from concourse.bass_utils import run_bass_kernel_spmd

        nc = _get_nc()
        res = run_bass_kernel_spmd(nc, in_maps, core_ids=list(range(NCORES)))
        return np.concatenate(
            [res.results[i]["out"] for i in range(NCORES)], axis=0
        )
